# revision 11
# baseline (speedup 1.0000x reference)
"""MoE routing kernel for Trainium2 (Bass/Tile), 8-core data-parallel.

Problem: out = einsum('be,beo->bo', softmax(x@Wg+bg, axis=1),
                      einsum('bd,edo->beo', x, We) + be)
with B=8192, D=1024, O=1024, E=8 (all experts dense, softmax-weighted).

Strategy (clustered mixture + fp8 correction):
  out_b = x_b @ M_c + sum_e dg_be * (x_b @ We_e) + g_b @ be
where rows are permuted host-side so each core's 1024 rows have similar
gate vectors (sorted by top-2 experts), c = per-core mean gate vector,
M_c = sum_e c_e We_e (host-precomputed, bf16), and dg = g - c is small
(|dg| ~ 0.14 << |g| ~ 0.4). The dominant term1 is ONE dense bf16 GEMM
(1/8 of the naive expert compute); the correction runs in fp8 e4m3 with
DoubleRow perf mode (K=256 per instruction -> 2x MACs at the same
per-instruction cost, 157 TF/s measured) since its ~4% relative error
only touches the small dg-weighted residual. Gates are computed
on-device exactly as in the dense baseline; dg is formed on-chip from
the host-supplied centroid.

Per core:
  phase 1 (PSUM pools gps+bps, closed after):
    gates = softmax(x@Wg + bg); dg_s = g*2^-17 - c_s (DVE);
    gT via PE transpose (for the g@be bias matmul);
    term1 acc[m,n] = x@M + g@be accumulated in PSUM, copied to SBUF.
  phase 2 (PSUM pool eps = 4 expert tags x 2 bufs = all 8 banks):
    per (n,m) tile: 8 experts x 4 DoubleRow matmuls in two 4-expert
    halves; the stationary x_f8 k-pair is shared by the 4 experts of a
    half (amortizes LDWEIGHTS); combine acc += psum_e * dg_s[:,e] via
    fused DVE ops; DMA out. Double-buffered banks let the PE stream
    group i+1 while the DVE combines group i.

DMA issue cost (~600ns per dma_start) is spread across the scalar,
sync, gpsimd and vector queues so the scalar engine is free for gate
activations early; all host layouts are flat per-partition so each
tensor is a few large contiguous-run DMAs.
"""
from contextlib import ExitStack

import numpy as np
import ml_dtypes

import concourse.tile as tile
import concourse.mybir as mybir
from concourse import bacc
from concourse.bass_utils import run_bass_kernel_spmd
from concourse.masks import make_identity

B, D, O, E = 8192, 1024, 1024, 8
NCORES = 8
BS = B // NCORES          # batch rows per core
P = 128                   # partition dim
NT = 512                  # matmul moving free-dim / PSUM bank width (fp32)
KC = D // P               # contraction chunks (8)
KP = KC // 2              # DoubleRow k-chunk pairs (4)
MC = BS // P              # batch-row chunks per core (8)
NCH = O // NT             # output column chunks (2)

XS = 32.0                 # x fp8 scale (|x|max ~5.5 -> 176 < 240)
WS = 4096.0               # We fp8 scale (1/32 -> 128 < 240)
DEQ = 1.0 / (XS * WS)     # dequant constant folded into dg

F32 = mybir.dt.float32
BF16 = mybir.dt.bfloat16
F8 = mybir.dt.float8e4
MULT = mybir.AluOpType.mult
ADD = mybir.AluOpType.add
SUB = mybir.AluOpType.subtract
DR = mybir.MatmulPerfMode.DoubleRow


def _emit(nc, tc, xT, xTf8, Mmix, Wef8, Wg, bg, be, cs, out):
    ctx = ExitStack()
    with ctx:
        const = ctx.enter_context(tc.tile_pool(name="const", bufs=1))
        xp = ctx.enter_context(tc.tile_pool(name="xp", bufs=1))
        wp = ctx.enter_context(tc.tile_pool(name="wp", bufs=1))
        gp = ctx.enter_context(tc.tile_pool(name="gp", bufs=1))
        accp = ctx.enter_context(tc.tile_pool(name="accp", bufs=1))
        small = ctx.enter_context(tc.tile_pool(name="small", bufs=2))

        # ---- loads ----
        # xT first on both HWDGE queues (the PE's first real work, gates,
        # needs every xT k-chunk); gate constants + the mixture matrix
        # follow; bulk fp8 streams after (n=0 halves before n=1). The
        # gpsimd queue is a slow software queue -- never route bulk data
        # there.
        # xT is m-row-major ([p, (m*KC+k)*P + r] = x[m*P+r, k*P+p]) so
        # gate group m only waits on its own 256KB chunk; chunks for
        # consecutive m alternate queues, with the mixture-matrix
        # quarters interleaved so term1 can start right after the gates.
        wg_all = const.tile([P, KC * E], BF16, name="wg_all")
        nc.scalar.dma_start(wg_all[:], Wg)
        bg_sb = const.tile([1, E], F32, name="bg_sb")
        nc.scalar.dma_start(bg_sb[:], bg)
        cs_sb = const.tile([P, E], F32, name="cs_sb")
        nc.scalar.dma_start(cs_sb[:], cs)

        xt_all = xp.tile([P, MC * KC * P], BF16, name="xt_all")
        m_all = wp.tile([P, NCH, KC, NT], BF16, name="m_all")
        XMQ = KC * P
        KH = KC // 2

        def xt_chunk(m):
            eng = nc.scalar if m % 2 == 0 else nc.sync
            eng.dma_start(xt_all[:, m * XMQ:(m + 1) * XMQ],
                          xT[:, m * XMQ:(m + 1) * XMQ])

        def m_quarter(n, h):
            eng = nc.scalar if n == 0 else nc.sync
            eng.dma_start(
                m_all[:, n, h * KH:(h + 1) * KH, :]
                .rearrange("p k o -> p (k o)"),
                Mmix[n][:, h * KH * NT:(h + 1) * KH * NT])

        for m in range(4):
            xt_chunk(m)
        m_quarter(0, 0)
        m_quarter(1, 0)
        for m in range(4, MC):
            xt_chunk(m)
        m_quarter(0, 1)
        m_quarter(1, 1)
        be_sb = const.tile([E, O], BF16, name="be_sb")
        nc.scalar.dma_start(be_sb[:], be)

        # even experts (both n-halves, n0 first) on the sync queue now;
        # odd experts + xf8 are issued by the scalar engine AFTER the
        # gate-phase emission so the gate ACTIVATEs aren't stuck behind
        # a 30us burst of blocking dma_start issues.
        xf8_all = xp.tile([P, KC, BS], F8, name="xf8_all")
        we_all = []
        for e in range(E):
            t = wp.tile([P, NCH, KC, NT], F8, name=f"we{e}", tag=f"we{e}")
            we_all.append(t)
        for n in range(NCH):
            for e in range(0, E, 2):
                nc.sync.dma_start(
                    we_all[e][:, n, :, :].rearrange("p k o -> p (k o)"),
                    Wef8[e, n])

        ones_sb = const.tile([1, P], F32, name="ones_sb")
        nc.vector.memset(ones_sb[:], 1.0)
        ident = const.tile([P, P], F32, name="ident")
        make_identity(nc, ident[:])

        def xt(k, ms):
            base = (ms.start // P * KC + k) * P
            return xt_all[:, base:base + P]

        def wg(k):
            return wg_all[:, k * E:(k + 1) * E]

        warm_sb = const.tile([P, NT], BF16, name="warm_sb")
        nc.vector.memset(warm_sb[:], 0.0)

        dgs = []
        accs = {}
        gT_all = gp.tile([E, BS], BF16, name="gT_all")

        # ---- phase 1: gates + term1 (own PSUM pools, freed after) ----
        with tc.tile_pool(name="gps", bufs=2, space="PSUM") as gps, \
             tc.tile_pool(name="bps", bufs=2, space="PSUM") as bps:

            def warmup(n):
                for _ in range(n):
                    pwu = bps.tile([P, NT], F32, name="pwu", tag="pb0")
                    nc.tensor.matmul(pwu[:], warm_sb[:, :P], warm_sb[:],
                                     start=True, stop=True)

            warmup(8)

            # gates: softmax(x @ Wg + bg), dg_s, gT
            for m in range(MC):
                ms = slice(m * P, (m + 1) * P)
                pg = gps.tile([P, E], F32, name="pg", tag="pg")
                for k in range(KC):
                    nc.tensor.matmul(pg[:], xt(k, ms), wg(k),
                                     start=(k == 0), stop=False)
                nc.tensor.matmul(pg[:], ones_sb[:], bg_sb[:],
                                 start=False, stop=True)

                # no max-subtraction: logits are bounded (|logit| < ~3)
                g = gp.tile([P, E], F32, name=f"g{m}", tag=f"g{m}")
                den = small.tile([P, 1], F32, name="den", tag="den")
                nc.scalar.activation(g[:], pg[:],
                                     mybir.ActivationFunctionType.Exp,
                                     bias=0.0, scale=1.0, accum_out=den[:])
                rden = small.tile([P, 1], F32, name="rden", tag="rden")
                nc.vector.reciprocal(rden[:], den[:])
                nc.vector.tensor_scalar_mul(g[:], g[:], rden[:])

                # dg_s = g * DEQ - c*DEQ  (dequant scale folded in)
                dg = gp.tile([P, E], F32, name=f"dg{m}", tag=f"dg{m}")
                nc.vector.scalar_tensor_tensor(dg[:], g[:], DEQ, cs_sb[:],
                                               MULT, SUB)
                dgs.append(dg)

                pt = bps.tile([E, P], F32, name="pt", tag="pb0")
                nc.tensor.transpose(pt[:], g[:], ident[:])
                nc.vector.tensor_scalar_mul(gT_all[:, ms], pt[:], 1.0)

            # late scalar-queue issues: xf8 + odd experts (n0 first).
            # Emitted after the gate-phase scalar ops so the ACTIVATEs
            # run first; data lands well before phase 2 needs it.
            XH = KC // 2
            nc.scalar.dma_start(
                xf8_all[:, :XH, :].rearrange("p k b -> p (k b)"),
                xTf8[:, :XH * BS])
            nc.scalar.dma_start(
                xf8_all[:, XH:, :].rearrange("p k b -> p (k b)"),
                xTf8[:, XH * BS:])
            for n in range(NCH):
                for e in range(1, E, 2):
                    nc.scalar.dma_start(
                        we_all[e][:, n, :, :].rearrange("p k o -> p (k o)"),
                        Wef8[e, n])

            # term1 acc[m,n] = x @ M + g @ be; both n-chunks share each
            # stationary load (xt k-chunk / gT), halving LDWEIGHTS bubbles
            for m in range(MC):
                ms = slice(m * P, (m + 1) * P)
                pt1 = [bps.tile([P, NT], F32, name=f"pt1_{n}", tag=f"pb{n}")
                       for n in range(NCH)]
                for k in range(KC):
                    for n in range(NCH):
                        nc.tensor.matmul(pt1[n][:], xt(k, ms),
                                         m_all[:, n, k, :],
                                         start=(k == 0), stop=False)
                for n in range(NCH):
                    ns = slice(n * NT, (n + 1) * NT)
                    nc.tensor.matmul(pt1[n][:], gT_all[:, ms],
                                     be_sb[:, ns], start=False, stop=True)
                    acc = accp.tile([P, NT], F32, name=f"acc{m}_{n}",
                                    tag=f"acc{m}_{n}")
                    nc.scalar.copy(acc[:], pt1[n][:])
                    accs[(m, n)] = acc

        # ---- phase 2: fp8 DoubleRow corrections (all 8 PSUM banks) ----
        # Per (n,m): experts in two halves of 4; within a half the
        # stationary x_f8 k-pair is shared by all 4 experts; each expert
        # accumulates K=1024 over 4 DR matmuls. bufs=2 double-buffers the
        # banks so the PE streams group i+1 while the DVE combines i.
        with tc.tile_pool(name="eps", bufs=2, space="PSUM") as eps:
            for n in range(NCH):
                ns = slice(n * NT, (n + 1) * NT)
                for m in range(MC):
                    ms = slice(m * P, (m + 1) * P)
                    acc = accs[(m, n)]
                    for half in range(2):
                        pes = {}
                        for kk in range(KP):
                            for j in range(4):
                                e = half * 4 + j
                                if kk == 0:
                                    pes[j] = eps.tile([P, NT], F32,
                                                      name=f"pe{j}",
                                                      tag=f"pe{j}")
                                nc.tensor.matmul(
                                    pes[j][:],
                                    xf8_all[:, 2 * kk:2 * kk + 2, ms],
                                    we_all[e][:, n, 2 * kk:2 * kk + 2, :],
                                    start=(kk == 0), stop=(kk == KP - 1),
                                    perf_mode=DR)
                        for j in range(4):
                            e = half * 4 + j
                            nc.vector.scalar_tensor_tensor(
                                acc[:], pes[j][:], dgs[m][:, e:e + 1],
                                acc[:], MULT, ADD)
                    nc.scalar.dma_start(out[ms, ns], acc[:])


_NC_CACHE = {}


def _build():
    if "nc" in _NC_CACHE:
        return _NC_CACHE["nc"]
    nc = bacc.Bacc("TRN2", target_bir_lowering=False, debug=False,
                   num_devices=NCORES)
    xT = nc.dram_tensor("xT", [P, KC * BS], BF16, kind="ExternalInput").ap()
    xTf8 = nc.dram_tensor("xTf8", [P, KC * BS], F8, kind="ExternalInput").ap()
    Mmix = nc.dram_tensor("Mmix", [NCH, P, KC * NT], BF16,
                          kind="ExternalInput").ap()
    Wef8 = nc.dram_tensor("Wef8", [E, NCH, P, KC * NT], F8,
                          kind="ExternalInput").ap()
    Wg_t = nc.dram_tensor("Wg", [P, KC * E], BF16, kind="ExternalInput").ap()
    bg_t = nc.dram_tensor("bg", [1, E], F32, kind="ExternalInput").ap()
    be_t = nc.dram_tensor("be", [E, O], BF16, kind="ExternalInput").ap()
    cs_t = nc.dram_tensor("cs", [P, E], F32, kind="ExternalInput").ap()
    out = nc.dram_tensor("out", [BS, O], F32, kind="ExternalOutput").ap()
    with tile.TileContext(nc) as tc:
        _emit(nc, tc, xT, xTf8, Mmix, Wef8, Wg_t, bg_t, be_t, cs_t, out)
    nc.compile()
    _NC_CACHE["nc"] = nc
    return nc


def _prep(x, Wg, bg, We, be):
    bf = ml_dtypes.bfloat16
    f8 = ml_dtypes.float8_e4m3
    x = np.asarray(x, dtype=np.float32)
    Wg32 = np.asarray(Wg, dtype=np.float32)
    bg32 = np.asarray(bg, dtype=np.float32).reshape(1, E)
    We32 = np.asarray(We, dtype=np.float32)
    be32 = np.asarray(be, dtype=np.float32)

    # host gates (routing metadata only; device recomputes gates exactly)
    logits = x @ Wg32 + bg32
    logits -= logits.max(axis=1, keepdims=True)
    g = np.exp(logits)
    g /= g.sum(axis=1, keepdims=True)
    srt = np.argsort(g, axis=1)
    order = np.lexsort((srt[:, -2], srt[:, -1]))
    inv = np.empty(B, np.int64)
    inv[order] = np.arange(B)

    xs = x[order]
    gs = g[order]

    # Wg re-laid out [p, k*E+e] = Wg[k*P+p, e] (one contiguous run per
    # partition instead of 16-byte rows)
    Wg_bf = np.ascontiguousarray(
        Wg32.astype(bf).reshape(KC, P, E).transpose(1, 0, 2)
        .reshape(P, KC * E))
    be_bf = be32.astype(bf)
    # We fp8, n-half major: Wef8[e, n, p, k*NT+o'] = We[e, k*P+p, n*NT+o']*WS
    We_f8 = np.ascontiguousarray(
        (We32 * WS).astype(f8).reshape(E, KC, P, NCH, NT)
        .transpose(0, 3, 2, 1, 4).reshape(E, NCH, P, KC * NT))

    maps = []
    for c in range(NCORES):
        xc = xs[c * BS:(c + 1) * BS]              # [BS, D]
        # xT_r[p, (m*KC+k)*P + r] = xc[m*P+r, k*P+p]  (m-row-major)
        xT = np.ascontiguousarray(
            xc.astype(bf).reshape(MC, P, KC, P).transpose(3, 0, 2, 1)
            .reshape(P, MC * KC * P))
        xTf8 = np.ascontiguousarray(
            (xc * XS).astype(f8).reshape(BS, KC, P).transpose(2, 1, 0)
            .reshape(P, KC * BS))
        cent = gs[c * BS:(c + 1) * BS].mean(axis=0).astype(np.float32)
        Mc = np.einsum('e,edo->do', cent, We32).astype(bf)
        # M n-half major: Mc_r[n, p, k*NT+o'] = Mc[k*P+p, n*NT+o']
        Mc = np.ascontiguousarray(
            Mc.reshape(KC, P, NCH, NT).transpose(2, 1, 0, 3)
            .reshape(NCH, P, KC * NT))
        cs = np.broadcast_to((cent * DEQ)[None, :], (P, E)).astype(np.float32)
        maps.append({"xT": xT, "xTf8": xTf8, "Mmix": Mc, "Wef8": We_f8,
                     "Wg": Wg_bf, "bg": bg32, "be": be_bf,
                     "cs": np.ascontiguousarray(cs)})
    return maps, inv


def run(x, Wg, bg, We, be, **spmd_kwargs):
    nc = _build()
    maps, inv = _prep(x, Wg, bg, We, be)
    res = run_bass_kernel_spmd(nc, maps, core_ids=list(range(NCORES)),
                               **spmd_kwargs)
    out = np.concatenate([res.results[c]["out"] for c in range(NCORES)],
                         axis=0)[inv]
    return out, res


def kernel(x, Wg, bg, We, be):
    out, _ = run(x, Wg, bg, We, be)
    return out


# revision 12
# speedup vs baseline: 1.0123x; 1.0123x over previous
"""MoE routing kernel for Trainium2 (Bass/Tile), 8-core data-parallel.

Problem: out = einsum('be,beo->bo', softmax(x@Wg+bg, axis=1),
                      einsum('bd,edo->beo', x, We) + be)
with B=8192, D=1024, O=1024, E=8 (all experts dense, softmax-weighted).

Strategy (clustered mixture + fp8 correction):
  out_b = x_b @ M_c + sum_e dg_be * (x_b @ We_e) + g_b @ be
where rows are permuted host-side so each core's 1024 rows have similar
gate vectors (sorted by top-2 experts), c = per-core mean gate vector,
M_c = sum_e c_e We_e (host-precomputed, bf16), and dg = g - c is small
(|dg| ~ 0.14 << |g| ~ 0.4). The dominant term1 is ONE dense bf16 GEMM
(1/8 of the naive expert compute); the correction runs in fp8 e4m3 with
DoubleRow perf mode (K=256 per instruction -> 2x MACs at the same
per-instruction cost, 157 TF/s measured) since its ~4% relative error
only touches the small dg-weighted residual. Gates are computed
on-device exactly as in the dense baseline; dg is formed on-chip from
the host-supplied centroid.

Per core:
  phase 1 (PSUM pools gps+bps, closed after):
    gates = softmax(x@Wg + bg); dg_s = g*2^-17 - c_s (DVE);
    gT via PE transpose (for the g@be bias matmul);
    term1 acc[m,n] = x@M + g@be accumulated in PSUM, copied to SBUF.
  phase 2 (PSUM pool eps = 4 expert tags x 2 bufs = all 8 banks):
    per (n,m) tile: 8 experts x 4 DoubleRow matmuls in two 4-expert
    halves; the stationary x_f8 k-pair is shared by the 4 experts of a
    half (amortizes LDWEIGHTS); combine acc += psum_e * dg_s[:,e] via
    fused DVE ops; DMA out. Double-buffered banks let the PE stream
    group i+1 while the DVE combines group i.

DMA issue cost (~600ns per dma_start) is spread across the scalar,
sync, gpsimd and vector queues so the scalar engine is free for gate
activations early; all host layouts are flat per-partition so each
tensor is a few large contiguous-run DMAs.
"""
from contextlib import ExitStack

import numpy as np
import ml_dtypes

import concourse.tile as tile
import concourse.mybir as mybir
from concourse import bacc
from concourse.bass_utils import run_bass_kernel_spmd
from concourse.masks import make_identity

B, D, O, E = 8192, 1024, 1024, 8
NCORES = 8
BS = B // NCORES          # batch rows per core
P = 128                   # partition dim
NT = 512                  # matmul moving free-dim / PSUM bank width (fp32)
KC = D // P               # contraction chunks (8)
KP = KC // 2              # DoubleRow k-chunk pairs (4)
MC = BS // P              # batch-row chunks per core (8)
NCH = O // NT             # output column chunks (2)

XS = 32.0                 # x fp8 scale (|x|max ~5.5 -> 176 < 240)
WS = 4096.0               # We fp8 scale (1/32 -> 128 < 240)
DEQ = 1.0 / (XS * WS)     # dequant constant folded into dg

F32 = mybir.dt.float32
BF16 = mybir.dt.bfloat16
F8 = mybir.dt.float8e4
MULT = mybir.AluOpType.mult
ADD = mybir.AluOpType.add
SUB = mybir.AluOpType.subtract
DR = mybir.MatmulPerfMode.DoubleRow


def _emit(nc, tc, xT, xTf8, Mmix, Wef8, Wg, bg, be, cs, out):
    ctx = ExitStack()
    with ctx:
        const = ctx.enter_context(tc.tile_pool(name="const", bufs=1))
        xp = ctx.enter_context(tc.tile_pool(name="xp", bufs=1))
        wp = ctx.enter_context(tc.tile_pool(name="wp", bufs=1))
        gp = ctx.enter_context(tc.tile_pool(name="gp", bufs=1))
        accp = ctx.enter_context(tc.tile_pool(name="accp", bufs=1))
        small = ctx.enter_context(tc.tile_pool(name="small", bufs=2))

        # ---- loads ----
        # xT first on both HWDGE queues (the PE's first real work, gates,
        # needs every xT k-chunk); gate constants + the mixture matrix
        # follow; bulk fp8 streams after (n=0 halves before n=1). The
        # gpsimd queue is a slow software queue -- never route bulk data
        # there.
        # xT is m-row-major ([p, (m*KC+k)*P + r] = x[m*P+r, k*P+p]) so
        # gate group m only waits on its own 256KB chunk; chunks for
        # consecutive m alternate queues, with the mixture-matrix
        # quarters interleaved so term1 can start right after the gates.
        wg_all = const.tile([P, KC * E], BF16, name="wg_all")
        nc.scalar.dma_start(wg_all[:], Wg)
        bg_sb = const.tile([1, E], F32, name="bg_sb")
        nc.scalar.dma_start(bg_sb[:], bg)
        cs_sb = const.tile([P, E], F32, name="cs_sb")
        nc.scalar.dma_start(cs_sb[:], cs)

        xt_all = xp.tile([P, MC * KC * P], BF16, name="xt_all")
        m_all = wp.tile([P, NCH, KC, NT], BF16, name="m_all")
        XMQ = KC * P
        KH = KC // 2

        def xt_chunk(m):
            eng = nc.scalar if m % 2 == 0 else nc.sync
            eng.dma_start(xt_all[:, m * XMQ:(m + 1) * XMQ],
                          xT[:, m * XMQ:(m + 1) * XMQ])

        def m_quarter(n, h):
            eng = nc.scalar if n == 0 else nc.sync
            eng.dma_start(
                m_all[:, n, h * KH:(h + 1) * KH, :]
                .rearrange("p k o -> p (k o)"),
                Mmix[n][:, h * KH * NT:(h + 1) * KH * NT])

        for m in range(4):
            xt_chunk(m)
        m_quarter(0, 0)
        m_quarter(1, 0)
        for m in range(4, MC):
            xt_chunk(m)
        m_quarter(0, 1)
        m_quarter(1, 1)
        be_sb = const.tile([E, O], BF16, name="be_sb")
        nc.scalar.dma_start(be_sb[:], be)

        # even experts (both n-halves, n0 first) on the sync queue now;
        # odd experts + xf8 are issued by the scalar engine AFTER the
        # gate-phase emission so the gate ACTIVATEs aren't stuck behind
        # a 30us burst of blocking dma_start issues.
        xf8_all = xp.tile([P, KC, BS], F8, name="xf8_all")
        we_all = []
        for e in range(E):
            t = wp.tile([P, NCH, KC, NT], F8, name=f"we{e}", tag=f"we{e}")
            we_all.append(t)
        for n in range(NCH):
            for e in range(0, E, 2):
                nc.sync.dma_start(
                    we_all[e][:, n, :, :].rearrange("p k o -> p (k o)"),
                    Wef8[e, n])

        ones_sb = const.tile([1, P], F32, name="ones_sb")
        nc.vector.memset(ones_sb[:], 1.0)
        ident = const.tile([P, P], F32, name="ident")
        make_identity(nc, ident[:])

        def xt(k, ms):
            base = (ms.start // P * KC + k) * P
            return xt_all[:, base:base + P]

        def wg(k):
            return wg_all[:, k * E:(k + 1) * E]

        warm_sb = const.tile([P, NT], BF16, name="warm_sb")
        nc.vector.memset(warm_sb[:], 0.0)

        dgs = []
        accs = {}
        gT_all = gp.tile([E, BS], BF16, name="gT_all")

        # ---- phase 1: gates + term1 (own PSUM pools, freed after) ----
        with tc.tile_pool(name="gps", bufs=2, space="PSUM") as gps, \
             tc.tile_pool(name="bps", bufs=2, space="PSUM") as bps:

            def warmup(n):
                for _ in range(n):
                    pwu = bps.tile([P, NT], F32, name="pwu", tag="pb0")
                    nc.tensor.matmul(pwu[:], warm_sb[:, :P], warm_sb[:],
                                     start=True, stop=True)

            warmup(8)

            # gates: softmax(x @ Wg + bg), dg_s, gT
            for m in range(MC):
                ms = slice(m * P, (m + 1) * P)
                pg = gps.tile([P, E], F32, name="pg", tag="pg")
                for k in range(KC):
                    nc.tensor.matmul(pg[:], xt(k, ms), wg(k),
                                     start=(k == 0), stop=False)
                nc.tensor.matmul(pg[:], ones_sb[:], bg_sb[:],
                                 start=False, stop=True)

                # no max-subtraction: logits are bounded (|logit| < ~3)
                g = gp.tile([P, E], F32, name=f"g{m}", tag=f"g{m}")
                den = small.tile([P, 1], F32, name="den", tag="den")
                nc.scalar.activation(g[:], pg[:],
                                     mybir.ActivationFunctionType.Exp,
                                     bias=0.0, scale=1.0, accum_out=den[:])
                rden = small.tile([P, 1], F32, name="rden", tag="rden")
                nc.vector.reciprocal(rden[:], den[:])
                nc.vector.tensor_scalar_mul(g[:], g[:], rden[:])

                # dg_s = g * DEQ - c*DEQ  (dequant scale folded in)
                dg = gp.tile([P, E], F32, name=f"dg{m}", tag=f"dg{m}")
                nc.vector.scalar_tensor_tensor(dg[:], g[:], DEQ, cs_sb[:],
                                               MULT, SUB)
                dgs.append(dg)

                pt = bps.tile([E, P], F32, name="pt", tag="pb0")
                nc.tensor.transpose(pt[:], g[:], ident[:])
                nc.vector.tensor_scalar_mul(gT_all[:, ms], pt[:], 1.0)

            # late scalar-queue issues: xf8 + odd experts (n0 first).
            # Emitted after the gate-phase scalar ops so the ACTIVATEs
            # run first; data lands well before phase 2 needs it.
            XH = KC // 2
            nc.scalar.dma_start(
                xf8_all[:, :XH, :].rearrange("p k b -> p (k b)"),
                xTf8[:, :XH * BS])
            nc.scalar.dma_start(
                xf8_all[:, XH:, :].rearrange("p k b -> p (k b)"),
                xTf8[:, XH * BS:])
            for n in range(NCH):
                for e in range(1, E, 2):
                    nc.scalar.dma_start(
                        we_all[e][:, n, :, :].rearrange("p k o -> p (k o)"),
                        Wef8[e, n])

            # term1 acc[m,n] = x @ M + g @ be; both n-chunks share each
            # stationary load (xt k-chunk / gT), halving LDWEIGHTS bubbles
            for m in range(MC):
                ms = slice(m * P, (m + 1) * P)
                pt1 = [bps.tile([P, NT], F32, name=f"pt1_{n}", tag=f"pb{n}")
                       for n in range(NCH)]
                for k in range(KC):
                    for n in range(NCH):
                        nc.tensor.matmul(pt1[n][:], xt(k, ms),
                                         m_all[:, n, k, :],
                                         start=(k == 0), stop=False)
                for n in range(NCH):
                    ns = slice(n * NT, (n + 1) * NT)
                    nc.tensor.matmul(pt1[n][:], gT_all[:, ms],
                                     be_sb[:, ns], start=False, stop=True)
                    acc = accp.tile([P, NT], F32, name=f"acc{m}_{n}",
                                    tag=f"acc{m}_{n}")
                    # vector, not scalar: the scalar engine is busy issuing
                    # the post-gate DMA burst and would stall bank recycling
                    nc.vector.tensor_scalar_mul(acc[:], pt1[n][:], 1.0)
                    accs[(m, n)] = acc

        # ---- phase 2: fp8 DoubleRow corrections (all 8 PSUM banks) ----
        # Per (n,m): experts in two halves of 4; within a half the
        # stationary x_f8 k-pair is shared by all 4 experts; each expert
        # accumulates K=1024 over 4 DR matmuls. bufs=2 double-buffers the
        # banks so the PE streams group i+1 while the DVE combines i.
        with tc.tile_pool(name="eps", bufs=2, space="PSUM") as eps:
            for n in range(NCH):
                ns = slice(n * NT, (n + 1) * NT)
                for m in range(MC):
                    ms = slice(m * P, (m + 1) * P)
                    acc = accs[(m, n)]
                    for half in range(2):
                        pes = {}
                        for kk in range(KP):
                            for j in range(4):
                                e = half * 4 + j
                                if kk == 0:
                                    pes[j] = eps.tile([P, NT], F32,
                                                      name=f"pe{j}",
                                                      tag=f"pe{j}")
                                nc.tensor.matmul(
                                    pes[j][:],
                                    xf8_all[:, 2 * kk:2 * kk + 2, ms],
                                    we_all[e][:, n, 2 * kk:2 * kk + 2, :],
                                    start=(kk == 0), stop=(kk == KP - 1),
                                    perf_mode=DR)
                        for j in range(4):
                            e = half * 4 + j
                            nc.vector.scalar_tensor_tensor(
                                acc[:], pes[j][:], dgs[m][:, e:e + 1],
                                acc[:], MULT, ADD)
                    nc.scalar.dma_start(out[ms, ns], acc[:])


_NC_CACHE = {}


def _build():
    if "nc" in _NC_CACHE:
        return _NC_CACHE["nc"]
    nc = bacc.Bacc("TRN2", target_bir_lowering=False, debug=False,
                   num_devices=NCORES)
    xT = nc.dram_tensor("xT", [P, KC * BS], BF16, kind="ExternalInput").ap()
    xTf8 = nc.dram_tensor("xTf8", [P, KC * BS], F8, kind="ExternalInput").ap()
    Mmix = nc.dram_tensor("Mmix", [NCH, P, KC * NT], BF16,
                          kind="ExternalInput").ap()
    Wef8 = nc.dram_tensor("Wef8", [E, NCH, P, KC * NT], F8,
                          kind="ExternalInput").ap()
    Wg_t = nc.dram_tensor("Wg", [P, KC * E], BF16, kind="ExternalInput").ap()
    bg_t = nc.dram_tensor("bg", [1, E], F32, kind="ExternalInput").ap()
    be_t = nc.dram_tensor("be", [E, O], BF16, kind="ExternalInput").ap()
    cs_t = nc.dram_tensor("cs", [P, E], F32, kind="ExternalInput").ap()
    out = nc.dram_tensor("out", [BS, O], F32, kind="ExternalOutput").ap()
    with tile.TileContext(nc) as tc:
        _emit(nc, tc, xT, xTf8, Mmix, Wef8, Wg_t, bg_t, be_t, cs_t, out)
    nc.compile()
    _NC_CACHE["nc"] = nc
    return nc


def _prep(x, Wg, bg, We, be):
    bf = ml_dtypes.bfloat16
    f8 = ml_dtypes.float8_e4m3
    x = np.asarray(x, dtype=np.float32)
    Wg32 = np.asarray(Wg, dtype=np.float32)
    bg32 = np.asarray(bg, dtype=np.float32).reshape(1, E)
    We32 = np.asarray(We, dtype=np.float32)
    be32 = np.asarray(be, dtype=np.float32)

    # host gates (routing metadata only; device recomputes gates exactly)
    logits = x @ Wg32 + bg32
    logits -= logits.max(axis=1, keepdims=True)
    g = np.exp(logits)
    g /= g.sum(axis=1, keepdims=True)
    srt = np.argsort(g, axis=1)
    order = np.lexsort((srt[:, -2], srt[:, -1]))
    inv = np.empty(B, np.int64)
    inv[order] = np.arange(B)

    xs = x[order]
    gs = g[order]

    # Wg re-laid out [p, k*E+e] = Wg[k*P+p, e] (one contiguous run per
    # partition instead of 16-byte rows)
    Wg_bf = np.ascontiguousarray(
        Wg32.astype(bf).reshape(KC, P, E).transpose(1, 0, 2)
        .reshape(P, KC * E))
    be_bf = be32.astype(bf)
    # We fp8, n-half major: Wef8[e, n, p, k*NT+o'] = We[e, k*P+p, n*NT+o']*WS
    We_f8 = np.ascontiguousarray(
        (We32 * WS).astype(f8).reshape(E, KC, P, NCH, NT)
        .transpose(0, 3, 2, 1, 4).reshape(E, NCH, P, KC * NT))

    maps = []
    for c in range(NCORES):
        xc = xs[c * BS:(c + 1) * BS]              # [BS, D]
        # xT_r[p, (m*KC+k)*P + r] = xc[m*P+r, k*P+p]  (m-row-major)
        xT = np.ascontiguousarray(
            xc.astype(bf).reshape(MC, P, KC, P).transpose(3, 0, 2, 1)
            .reshape(P, MC * KC * P))
        xTf8 = np.ascontiguousarray(
            (xc * XS).astype(f8).reshape(BS, KC, P).transpose(2, 1, 0)
            .reshape(P, KC * BS))
        cent = gs[c * BS:(c + 1) * BS].mean(axis=0).astype(np.float32)
        Mc = np.einsum('e,edo->do', cent, We32).astype(bf)
        # M n-half major: Mc_r[n, p, k*NT+o'] = Mc[k*P+p, n*NT+o']
        Mc = np.ascontiguousarray(
            Mc.reshape(KC, P, NCH, NT).transpose(2, 1, 0, 3)
            .reshape(NCH, P, KC * NT))
        cs = np.broadcast_to((cent * DEQ)[None, :], (P, E)).astype(np.float32)
        maps.append({"xT": xT, "xTf8": xTf8, "Mmix": Mc, "Wef8": We_f8,
                     "Wg": Wg_bf, "bg": bg32, "be": be_bf,
                     "cs": np.ascontiguousarray(cs)})
    return maps, inv


def run(x, Wg, bg, We, be, **spmd_kwargs):
    nc = _build()
    maps, inv = _prep(x, Wg, bg, We, be)
    res = run_bass_kernel_spmd(nc, maps, core_ids=list(range(NCORES)),
                               **spmd_kwargs)
    out = np.concatenate([res.results[c]["out"] for c in range(NCORES)],
                         axis=0)[inv]
    return out, res


def kernel(x, Wg, bg, We, be):
    out, _ = run(x, Wg, bg, We, be)
    return out


# revision 13
# speedup vs baseline: 1.0686x; 1.0557x over previous
"""MoE routing kernel for Trainium2 (Bass/Tile), 8-core data-parallel.

Problem: out = einsum('be,beo->bo', softmax(x@Wg+bg, axis=1),
                      einsum('bd,edo->beo', x, We) + be)
with B=8192, D=1024, O=1024, E=8 (all experts dense, softmax-weighted).

Strategy (clustered mixture + fp8 correction):
  out_b = x_b @ M_c + sum_e dg_be * (x_b @ We_e) + g_b @ be
where rows are permuted host-side so each core's 1024 rows have similar
gate vectors (sorted by top-2 experts), c = per-core mean gate vector,
M_c = sum_e c_e We_e (host-precomputed, bf16), and dg = g - c is small
(|dg| ~ 0.14 << |g| ~ 0.4). The dominant term1 is ONE dense bf16 GEMM
(1/8 of the naive expert compute); the correction runs in fp8 e4m3 with
DoubleRow perf mode (K=256 per instruction -> 2x MACs at the same
per-instruction cost, 157 TF/s measured) since its ~4% relative error
only touches the small dg-weighted residual. Gates are computed
on-device exactly as in the dense baseline; dg is formed on-chip from
the host-supplied centroid.

Per core:
  phase 1 (PSUM pools gps+bps, closed after):
    gates = softmax(x@Wg + bg); dg_s = g*2^-17 - c_s (DVE);
    gT via PE transpose (for the g@be bias matmul);
    term1 acc[m,n] = x@M + g@be accumulated in PSUM, copied to SBUF.
  phase 2 (PSUM pool eps = 4 expert tags x 2 bufs = all 8 banks):
    per (n,m) tile: 8 experts x 4 DoubleRow matmuls in two 4-expert
    halves; the stationary x_f8 k-pair is shared by the 4 experts of a
    half (amortizes LDWEIGHTS); combine acc += psum_e * dg_s[:,e] via
    fused DVE ops; DMA out. Double-buffered banks let the PE stream
    group i+1 while the DVE combines group i.

DMA issue cost (~600ns per dma_start) is spread across the scalar,
sync, gpsimd and vector queues so the scalar engine is free for gate
activations early; all host layouts are flat per-partition so each
tensor is a few large contiguous-run DMAs.
"""
from contextlib import ExitStack

import numpy as np
import ml_dtypes

import concourse.tile as tile
import concourse.mybir as mybir
from concourse import bacc
from concourse.bass_utils import run_bass_kernel_spmd
from concourse.masks import make_identity

B, D, O, E = 8192, 1024, 1024, 8
NCORES = 8
BS = B // NCORES          # batch rows per core
P = 128                   # partition dim
NT = 512                  # matmul moving free-dim / PSUM bank width (fp32)
KC = D // P               # contraction chunks (8)
KP = KC // 2              # DoubleRow k-chunk pairs (4)
MC = BS // P              # batch-row chunks per core (8)
NCH = O // NT             # output column chunks (2)

XS = 32.0                 # x fp8 scale (|x|max ~5.5 -> 176 < 240)
WS = 4096.0               # We fp8 scale (1/32 -> 128 < 240)
DEQ = 1.0 / (XS * WS)     # dequant constant folded into dg

F32 = mybir.dt.float32
BF16 = mybir.dt.bfloat16
F8 = mybir.dt.float8e4
MULT = mybir.AluOpType.mult
ADD = mybir.AluOpType.add
SUB = mybir.AluOpType.subtract
DR = mybir.MatmulPerfMode.DoubleRow


def _emit(nc, tc, xT, xTf8, Mmix, Wef8, Wg, bg, be, cs, out):
    ctx = ExitStack()
    with ctx:
        const = ctx.enter_context(tc.tile_pool(name="const", bufs=1))
        xp = ctx.enter_context(tc.tile_pool(name="xp", bufs=1))
        wp = ctx.enter_context(tc.tile_pool(name="wp", bufs=1))
        gp = ctx.enter_context(tc.tile_pool(name="gp", bufs=1))
        accp = ctx.enter_context(tc.tile_pool(name="accp", bufs=1))
        small = ctx.enter_context(tc.tile_pool(name="small", bufs=2))

        # ---- loads ----
        # xT first on both HWDGE queues (the PE's first real work, gates,
        # needs every xT k-chunk); gate constants + the mixture matrix
        # follow; bulk fp8 streams after (n=0 halves before n=1). The
        # gpsimd queue is a slow software queue -- never route bulk data
        # there.
        # xT is m-row-major ([p, (m*KC+k)*P + r] = x[m*P+r, k*P+p]) so
        # gate group m only waits on its own 256KB chunk; chunks for
        # consecutive m alternate queues, with the mixture-matrix
        # quarters interleaved so term1 can start right after the gates.
        wg_all = const.tile([P, KC * E], BF16, name="wg_all")
        nc.scalar.dma_start(wg_all[:], Wg)
        bg_sb = const.tile([1, E], F32, name="bg_sb")
        nc.scalar.dma_start(bg_sb[:], bg)
        cs_sb = const.tile([P, E], F32, name="cs_sb")
        nc.scalar.dma_start(cs_sb[:], cs)

        xt_all = xp.tile([P, MC * KC * P], BF16, name="xt_all")
        m_all = wp.tile([P, NCH, KC, NT], BF16, name="m_all")
        XMQ = KC * P
        KH = KC // 2

        def xt_chunk(m):
            eng = nc.scalar if m % 2 == 0 else nc.sync
            eng.dma_start(xt_all[:, m * XMQ:(m + 1) * XMQ],
                          xT[:, m * XMQ:(m + 1) * XMQ])

        def m_quarter(n, h):
            eng = nc.scalar if n == 0 else nc.sync
            eng.dma_start(
                m_all[:, n, h * KH:(h + 1) * KH, :]
                .rearrange("p k o -> p (k o)"),
                Mmix[n][:, h * KH * NT:(h + 1) * KH * NT])

        for m in range(4):
            xt_chunk(m)
        m_quarter(0, 0)
        m_quarter(1, 0)
        for m in range(4, MC):
            xt_chunk(m)
        m_quarter(0, 1)
        m_quarter(1, 1)
        be_sb = const.tile([E, O], BF16, name="be_sb")
        nc.scalar.dma_start(be_sb[:], be)

        # even experts (both n-halves, n0 first) on the sync queue now;
        # odd experts + xf8 are issued by the scalar engine AFTER the
        # gate-phase emission so the gate ACTIVATEs aren't stuck behind
        # a 30us burst of blocking dma_start issues.
        xf8_all = xp.tile([P, KC, BS], F8, name="xf8_all")
        we_all = []
        for e in range(E):
            t = wp.tile([P, NCH, KC, NT], F8, name=f"we{e}", tag=f"we{e}")
            we_all.append(t)
        for n in range(NCH):
            for e in range(0, E, 2):
                nc.sync.dma_start(
                    we_all[e][:, n, :, :].rearrange("p k o -> p (k o)"),
                    Wef8[e, n])

        ones_sb = const.tile([1, P], F32, name="ones_sb")
        nc.vector.memset(ones_sb[:], 1.0)
        ident = const.tile([P, P], F32, name="ident")
        make_identity(nc, ident[:])

        def xt(k, ms):
            base = (ms.start // P * KC + k) * P
            return xt_all[:, base:base + P]

        def wg(k):
            return wg_all[:, k * E:(k + 1) * E]

        warm_sb = const.tile([P, NT], BF16, name="warm_sb")
        nc.vector.memset(warm_sb[:], 0.0)

        dgs = []
        accs = {}
        gT_all = gp.tile([E, BS], BF16, name="gT_all")

        # ---- phase 1: fused gates + term1 (own PSUM pools, freed after) ----
        # Per m-group the k-loop issues THREE matmuls per stationary
        # xt(k,ms): term1 n0, term1 n1 and the gate logits -- one
        # LDWEIGHTS for all three. The softmax chain for group m runs on
        # scalar/vector during group m+1's k-loop; the transpose lands
        # mid-way through the next k-loop and the bias matmuls close the
        # PSUM groups right after it, so the PE never waits on softmax.
        # The post-gate bulk DMA issues are spread between the Exp ops.
        with tc.tile_pool(name="gps", bufs=2, space="PSUM") as gps, \
             tc.tile_pool(name="bps", bufs=2, space="PSUM") as bps:

            def warmup(n):
                for _ in range(n):
                    pwu = bps.tile([P, NT], F32, name="pwu", tag="pb0")
                    nc.tensor.matmul(pwu[:], warm_sb[:, :P], warm_sb[:],
                                     start=True, stop=True)

            warmup(12)

            # late scalar-queue bulk issues, doled out two per m-group so
            # the gate ACTIVATEs are never stuck behind an issue burst
            XH = KC // 2
            late_dmas = [
                lambda: nc.scalar.dma_start(
                    xf8_all[:, :XH, :].rearrange("p k b -> p (k b)"),
                    xTf8[:, :XH * BS]),
                lambda: nc.scalar.dma_start(
                    xf8_all[:, XH:, :].rearrange("p k b -> p (k b)"),
                    xTf8[:, XH * BS:]),
            ]
            for n in range(NCH):
                for e in range(1, E, 2):
                    late_dmas.append(lambda e=e, n=n: nc.scalar.dma_start(
                        we_all[e][:, n, :, :].rearrange("p k o -> p (k o)"),
                        Wef8[e, n]))

            pt1s = {}
            gs_ = {}

            def emit_transpose(m):
                ms = slice(m * P, (m + 1) * P)
                pt = gps.tile([E, P], F32, name="pt", tag="pt")
                nc.tensor.transpose(pt[:], gs_[m][:], ident[:])
                nc.vector.tensor_scalar_mul(gT_all[:, ms], pt[:], 1.0)

            def emit_bias(m):
                ms = slice(m * P, (m + 1) * P)
                for n in range(NCH):
                    ns = slice(n * NT, (n + 1) * NT)
                    nc.tensor.matmul(pt1s[m][n][:], gT_all[:, ms],
                                     be_sb[:, ns], start=False, stop=True)
                    acc = accp.tile([P, NT], F32, name=f"acc{m}_{n}",
                                    tag=f"acc{m}_{n}")
                    nc.vector.tensor_scalar_mul(acc[:], pt1s[m][n][:], 1.0)
                    accs[(m, n)] = acc

            prev = None
            for m in range(MC):
                ms = slice(m * P, (m + 1) * P)
                pt1 = [bps.tile([P, NT], F32, name=f"pt1_{n}", tag=f"pb{n}")
                       for n in range(NCH)]
                pt1s[m] = pt1
                pg = gps.tile([P, E], F32, name="pg", tag="pg")
                for k in range(KC):
                    if k == 4 and prev is not None:
                        emit_transpose(prev)
                    nc.tensor.matmul(pt1[0][:], xt(k, ms), m_all[:, 0, k, :],
                                     start=(k == 0), stop=False)
                    nc.tensor.matmul(pt1[1][:], xt(k, ms), m_all[:, 1, k, :],
                                     start=(k == 0), stop=False)
                    nc.tensor.matmul(pg[:], xt(k, ms), wg(k),
                                     start=(k == 0), stop=False)
                nc.tensor.matmul(pg[:], ones_sb[:], bg_sb[:],
                                 start=False, stop=True)

                # softmax chain (scalar + vector), overlaps next k-loop
                g = gp.tile([P, E], F32, name=f"g{m}", tag=f"g{m}")
                den = small.tile([P, 1], F32, name="den", tag="den")
                nc.scalar.activation(g[:], pg[:],
                                     mybir.ActivationFunctionType.Exp,
                                     bias=0.0, scale=1.0, accum_out=den[:])
                rden = small.tile([P, 1], F32, name="rden", tag="rden")
                nc.vector.reciprocal(rden[:], den[:])
                nc.vector.tensor_scalar_mul(g[:], g[:], rden[:])
                gs_[m] = g
                dg = gp.tile([P, E], F32, name=f"dg{m}", tag=f"dg{m}")
                nc.vector.scalar_tensor_tensor(dg[:], g[:], DEQ, cs_sb[:],
                                               MULT, SUB)
                dgs.append(dg)

                for _ in range(2):
                    if late_dmas:
                        late_dmas.pop(0)()
                if prev is not None:
                    emit_bias(prev)
                prev = m
            while late_dmas:
                late_dmas.pop(0)()
            emit_transpose(prev)
            emit_bias(prev)

        # ---- phase 2: fp8 DoubleRow corrections (all 8 PSUM banks) ----
        # Per (n,m): experts in two halves of 4; within a half the
        # stationary x_f8 k-pair is shared by all 4 experts; each expert
        # accumulates K=1024 over 4 DR matmuls. bufs=2 double-buffers the
        # banks so the PE streams group i+1 while the DVE combines i.
        with tc.tile_pool(name="eps", bufs=2, space="PSUM") as eps:
            for n in range(NCH):
                ns = slice(n * NT, (n + 1) * NT)
                for m in range(MC):
                    ms = slice(m * P, (m + 1) * P)
                    acc = accs[(m, n)]
                    for half in range(2):
                        pes = {}
                        for kk in range(KP):
                            for j in range(4):
                                e = half * 4 + j
                                if kk == 0:
                                    pes[j] = eps.tile([P, NT], F32,
                                                      name=f"pe{j}",
                                                      tag=f"pe{j}")
                                nc.tensor.matmul(
                                    pes[j][:],
                                    xf8_all[:, 2 * kk:2 * kk + 2, ms],
                                    we_all[e][:, n, 2 * kk:2 * kk + 2, :],
                                    start=(kk == 0), stop=(kk == KP - 1),
                                    perf_mode=DR)
                        for j in range(4):
                            e = half * 4 + j
                            nc.vector.scalar_tensor_tensor(
                                acc[:], pes[j][:], dgs[m][:, e:e + 1],
                                acc[:], MULT, ADD)
                    nc.scalar.dma_start(out[ms, ns], acc[:])


_NC_CACHE = {}


def _build():
    if "nc" in _NC_CACHE:
        return _NC_CACHE["nc"]
    nc = bacc.Bacc("TRN2", target_bir_lowering=False, debug=False,
                   num_devices=NCORES)
    xT = nc.dram_tensor("xT", [P, KC * BS], BF16, kind="ExternalInput").ap()
    xTf8 = nc.dram_tensor("xTf8", [P, KC * BS], F8, kind="ExternalInput").ap()
    Mmix = nc.dram_tensor("Mmix", [NCH, P, KC * NT], BF16,
                          kind="ExternalInput").ap()
    Wef8 = nc.dram_tensor("Wef8", [E, NCH, P, KC * NT], F8,
                          kind="ExternalInput").ap()
    Wg_t = nc.dram_tensor("Wg", [P, KC * E], BF16, kind="ExternalInput").ap()
    bg_t = nc.dram_tensor("bg", [1, E], F32, kind="ExternalInput").ap()
    be_t = nc.dram_tensor("be", [E, O], BF16, kind="ExternalInput").ap()
    cs_t = nc.dram_tensor("cs", [P, E], F32, kind="ExternalInput").ap()
    out = nc.dram_tensor("out", [BS, O], F32, kind="ExternalOutput").ap()
    with tile.TileContext(nc) as tc:
        _emit(nc, tc, xT, xTf8, Mmix, Wef8, Wg_t, bg_t, be_t, cs_t, out)
    nc.compile()
    _NC_CACHE["nc"] = nc
    return nc


def _prep(x, Wg, bg, We, be):
    bf = ml_dtypes.bfloat16
    f8 = ml_dtypes.float8_e4m3
    x = np.asarray(x, dtype=np.float32)
    Wg32 = np.asarray(Wg, dtype=np.float32)
    bg32 = np.asarray(bg, dtype=np.float32).reshape(1, E)
    We32 = np.asarray(We, dtype=np.float32)
    be32 = np.asarray(be, dtype=np.float32)

    # host gates (routing metadata only; device recomputes gates exactly)
    logits = x @ Wg32 + bg32
    logits -= logits.max(axis=1, keepdims=True)
    g = np.exp(logits)
    g /= g.sum(axis=1, keepdims=True)
    srt = np.argsort(g, axis=1)
    order = np.lexsort((srt[:, -2], srt[:, -1]))
    inv = np.empty(B, np.int64)
    inv[order] = np.arange(B)

    xs = x[order]
    gs = g[order]

    # Wg re-laid out [p, k*E+e] = Wg[k*P+p, e] (one contiguous run per
    # partition instead of 16-byte rows)
    Wg_bf = np.ascontiguousarray(
        Wg32.astype(bf).reshape(KC, P, E).transpose(1, 0, 2)
        .reshape(P, KC * E))
    be_bf = be32.astype(bf)
    # We fp8, n-half major: Wef8[e, n, p, k*NT+o'] = We[e, k*P+p, n*NT+o']*WS
    We_f8 = np.ascontiguousarray(
        (We32 * WS).astype(f8).reshape(E, KC, P, NCH, NT)
        .transpose(0, 3, 2, 1, 4).reshape(E, NCH, P, KC * NT))

    maps = []
    for c in range(NCORES):
        xc = xs[c * BS:(c + 1) * BS]              # [BS, D]
        # xT_r[p, (m*KC+k)*P + r] = xc[m*P+r, k*P+p]  (m-row-major)
        xT = np.ascontiguousarray(
            xc.astype(bf).reshape(MC, P, KC, P).transpose(3, 0, 2, 1)
            .reshape(P, MC * KC * P))
        xTf8 = np.ascontiguousarray(
            (xc * XS).astype(f8).reshape(BS, KC, P).transpose(2, 1, 0)
            .reshape(P, KC * BS))
        cent = gs[c * BS:(c + 1) * BS].mean(axis=0).astype(np.float32)
        Mc = np.einsum('e,edo->do', cent, We32).astype(bf)
        # M n-half major: Mc_r[n, p, k*NT+o'] = Mc[k*P+p, n*NT+o']
        Mc = np.ascontiguousarray(
            Mc.reshape(KC, P, NCH, NT).transpose(2, 1, 0, 3)
            .reshape(NCH, P, KC * NT))
        cs = np.broadcast_to((cent * DEQ)[None, :], (P, E)).astype(np.float32)
        maps.append({"xT": xT, "xTf8": xTf8, "Mmix": Mc, "Wef8": We_f8,
                     "Wg": Wg_bf, "bg": bg32, "be": be_bf,
                     "cs": np.ascontiguousarray(cs)})
    return maps, inv


def run(x, Wg, bg, We, be, **spmd_kwargs):
    nc = _build()
    maps, inv = _prep(x, Wg, bg, We, be)
    res = run_bass_kernel_spmd(nc, maps, core_ids=list(range(NCORES)),
                               **spmd_kwargs)
    out = np.concatenate([res.results[c]["out"] for c in range(NCORES)],
                         axis=0)[inv]
    return out, res


def kernel(x, Wg, bg, We, be):
    out, _ = run(x, Wg, bg, We, be)
    return out
